# revision 31
# baseline (speedup 1.0000x reference)
"""Adaptive wavelet transform (db8 DWT -> quantile threshold mask -> IDWT) on
Trainium2, 8 NeuronCores, batch-sharded (4 batches per core).

Self-contained: hardcoded shapes [32, 4096, 512]; db8 filter taps inlined.

The mask decision (cd^2 > quantile-threshold) must match the XLA-CPU reference
bit-exactly (one flipped coefficient costs ~5e-2 on the absmax metric, over the
2e-2 gate).  XLA CPU computes the DWT conv as a sequential fused-FMA chain over
the 16 taps.  Instead of emulating that chain for every element (6 elementwise
passes x 15 taps, the old 4.1ms bottleneck), this version:

  P1: PE banded matmuls produce BOTH ca and an approximate cd (abs err ~1e-5);
      cd is PE-transposed to channel-major SBUF.  x is also transposed
      (extended, symmetric-mirrored) and spilled to DRAM for window gathers.
  P2: bit-space ternary search on approx cd^2 finds the approx k-th order
      statistic v~ and its successor vnext.  All elements with cd^2 in
      (lo(1-d), vnext(1+d)] (d=5e-4 >> PE error) form a <=16-element/row
      candidate pool; everything outside the pool provably has the same mask
      bit under exact and approx values.  Pool positions are extracted with a
      payload trick (select(in-pool, idx+4096, 0) -> max8/match_replace/max8),
      their 16-tap windows gathered from DRAM via indirect DMA, and the exact
      fused-FMA chain is replayed ONLY for those windows (Dekker two-product /
      two-sum custom DVE ops, [128,16] tiles).  Exact v_k/v_{k+1} come from
      sorting the pool's exact squares (max8 descending) and selecting rank
      cT2-(k+1); thr replicates XLA's fma(v_k, 1-g, RN(v_{k+1}*g)) via Dekker.
      Exact mask bits for pool elements are scattered into an f16 flag tile
      (gpsimd local_scatter, per-partition indices); one fused DVE pass applies
      flag==1 -> keep, flag==2 -> zero, else bulk compare cd^2 > v~.
  P3: per 64-sample output chunk, IDWT matmul from the spilled
      ca + masked-cd rows, PSUM -> SBUF -> DRAM out.
      The ternary search runs only 10 of 19 levels; the band is anchored at the
      surviving [lo, hi] interval (hi = lo + d_rem - 1 in bit space) so the
      pool absorbs the unresolved ranks (max pool ~8 of 16 capacity).
"""
import numpy as np

B, T, C = 32, 4096, 512
FLEN = 16
W = 2055
TE = T + 2 * (FLEN - 1)   # 4126 extended (symmetric-pad) cols; conv reads 1..4124
NB = 4            # batches per core
NCORES = 8
NCT = C // 128    # 4 channel tiles per batch
DELTA = 5e-4      # candidate band halfwidth (rel); PE abs err ~1e-5 << d*thr
LVL = 10          # ternary levels actually run (band machinery absorbs the rest)
NCAND = 16        # pool capacity (2 rounds of max8)

DB8_DEC_LO = np.array([
    -0.00011747678400228192, 0.0006754494059985568, -0.0003917403729959771,
    -0.00487035299301066, 0.008746094047015655, 0.013981027917015516,
    -0.04408825393106472, -0.01736930100202211, 0.128747426620186,
    0.00047248457399797254, -0.2840155429624281, -0.015829105256023893,
    0.5853546836548691, 0.6756307362980128, 0.3128715909144659,
    0.05441584224308161], dtype=np.float32)
_signs = ((-1.0) ** np.arange(FLEN)).astype(np.float32)
KHI = (DB8_DEC_LO * _signs).astype(np.float32)   # conv taps for cd, j ascending
DEC_LO = DB8_DEC_LO.astype(np.float64)
REC_LO = DEC_LO[::-1].copy()
REC_HI = (DEC_LO * _signs.astype(np.float64))
DEC_HI = REC_HI[::-1].copy()


# ------------------------- host-side constant builders ----------------------

def _mirror(t):
    t = np.asarray(t)
    t = np.where(t < 0, -1 - t, t)
    t = np.where(t >= T, 2 * T - 1 - t, t)
    return t


def build_dwt_matrix(h):
    A = np.zeros((T, W), dtype=np.float64)
    for i in range(W):
        for j in range(FLEN):
            A[_mirror(1 + 2 * i - j), i] += h[j]
    return A


def build_idwt_matrix(rec):
    R = np.zeros((W, T), dtype=np.float64)
    for i in range(W):
        for t in range(max(0, 2 * i - 14), min(T, 2 * i + 2)):
            j = t + 14 - 2 * i
            if 0 <= j < FLEN:
                R[i, t] += rec[j]
    return R


_HC = None


def host_consts():
    global _HC
    if _HC is not None:
        return _HC
    A_lo = build_dwt_matrix(DEC_LO)
    A_hi = build_dwt_matrix(DEC_HI)
    R_lo = build_idwt_matrix(REC_LO)
    R_hi = build_idwt_matrix(REC_HI)
    c = {}

    def ab(o):
        cols = slice(64 * o, 64 * o + 64)
        return np.concatenate([A_lo[:, cols], A_hi[:, cols]], axis=1)  # [T,128]

    c["MB0"] = ab(0)[0:128].astype(np.float32)
    c["MBi"] = ab(1)[128:256].astype(np.float32)
    c["MAi"] = ab(1)[0:128].astype(np.float32)
    assert np.abs(c["MAi"][64:114]).max() == 0
    tl = np.concatenate([A_lo[:, 2048:W], np.zeros((T, 57)),
                         A_hi[:, 2048:W]], axis=1)  # [T, 71]
    assert np.abs(tl[:3968]).max() == 0
    c["MT"] = tl[3968:4096].astype(np.float32)

    supports = []
    for v in range(64):
        cols = slice(64 * v, 64 * v + 64)
        nz = np.nonzero(np.abs(R_lo[:, cols]).sum(1) + np.abs(R_hi[:, cols]).sum(1))[0]
        supports.append((int(nz[0]), int(nz[-1] + 1)))
    for v in range(64):
        assert supports[v] == (32 * v, min(32 * v + 39, W)), (v, supports[v])
    c["IDWT_SUPPORT"] = supports

    def rblk(v):
        i0, i1 = supports[v]
        cols = slice(64 * v, 64 * v + 64)
        blk = np.concatenate([R_lo[i0:i1, cols], np.zeros((25, 64)),
                              R_hi[i0:i1, cols]], axis=0)
        return np.ascontiguousarray(blk.astype(np.float32))  # [103, 64]

    c["RBi"] = rblk(1)
    for v in (0, 2, 33, 62, 63):
        assert np.array_equal(rblk(v), c["RBi"])
    rl64 = np.ascontiguousarray(c["RBi"][0:64])            # 39 lo rows + 25 zero
    rh64 = np.concatenate([c["RBi"][64:103], np.zeros((25, 64), np.float32)])
    c["RL_A"] = rl64
    c["RH_A"] = np.ascontiguousarray(rh64)
    c["IDENT"] = np.eye(128, dtype=np.float32)
    _HC = c
    return c


CONST_NAMES = ("MB0", "MBi", "MAi", "MT", "RL_A", "RH_A", "IDENT")


def quantile_host_params(q):
    q = np.float32(q)
    n = np.float32(W)
    pos = np.float32(q * (n - np.float32(1.0)))
    low = np.float32(np.floor(pos))
    g = np.float32(pos - low)
    lw = np.float32(np.float32(1.0) - g)
    return int(low), float(g), float(lw)


def bisect_schedule():
    d = np.float32(512.0).view(np.int32).item() + 1
    ts = []
    while d > 1:
        t = (d + 2) // 3
        ts.append(t)
        d = t
    return ts


def veltkamp_split(b):
    b = np.float32(b)
    t = np.float32(b * np.float32(4097.0))
    bhi = np.float32(t - np.float32(t - b))
    blo = np.float32(b - bhi)
    return float(bhi), float(blo)


# ----------------------------- custom DVE ops -------------------------------

_OPS = {}


def _register_ops():
    if _OPS:
        return _OPS
    import concourse.dve_ops as D
    from concourse.dve_spec import (Spec, Src0, Src1, C0, C1, C2, Zero, One,
                                    sq, select, lower, minn, eq, Idx)
    from concourse.dve_spec import _has_src1 as has_src1
    from concourse.dve_uop import DveOpSpec
    from operator import add as _add

    def reg(name, spec, subdim=False):
        if name in D._SUB_OPCODE_FOR_NAME:
            return next(o for o in D.OPS if o.name == name)
        row = max(D._SUB_OPCODE_FOR_NAME.values()) + 1
        assert row < 0x20
        D._SUB_OPCODE_FOR_NAME[name] = row
        shas = {}
        for ver in ("v3", "v4"):
            r = DveOpSpec(name=name, opcode=row, uops=lower(spec, ver=ver),
                          rd1_en=has_src1(spec))
            shas[ver] = r.sha(ver)
        op = D.DveOp(name, spec, subdim, uops_sha=shas)
        D.OPS.append(op)
        D.CUSTOM_DVE_SPECS[name] = spec
        return op

    f32 = np.float32

    def _count2_ref(in0, in1, s0, s1, imm2):
        x = in0.astype(np.float32) ** 2
        body = ((x <= s0).astype(np.float32) + (x <= s1) * np.float32(imm2))
        return body, body.reshape(body.shape[0], -1).sum(-1, keepdims=True)

    def _condmin_ref(in0, in1, s0, s1, imm2):
        x = in0.astype(np.float32) ** 2
        body = np.where(x > s0, x, np.float32(imm2)).astype(np.float32)
        return body, body.reshape(body.shape[0], -1).min(-1, keepdims=True)

    def _e1_ref(in0, in1, s0, s1, imm2):
        # in0=x, in1=xl, s0=h, s1=hh, imm2=hl; xh = x - xl (exact)
        xh = (in0 - in1).astype(f32)
        p = (f32(s0) * in0).astype(f32)
        t1 = (f32(s1) * xh).astype(f32)
        t2 = (t1 - p).astype(f32)
        t3 = (f32(s1) * in1).astype(f32)
        t4 = (t2 + t3).astype(f32)
        t5 = (f32(imm2) * xh).astype(f32)
        return (t4 + t5).astype(f32)

    def _axpy_ref(in0, in1, s0, s1, imm2):
        return (in0 + (f32(s0) * in1).astype(f32)).astype(f32)

    def _twosumt_ref(in0, in1, s0, s1, imm2):
        s = (in0 + in1).astype(f32)
        bb = (s - in0).astype(f32)
        u = (s - bb).astype(f32)
        v = (in0 - u).astype(f32)
        w = (in1 - bb).astype(f32)
        return (v + w).astype(f32)

    def _splitlo_ref(in0, in1, s0, s1, imm2):
        t = (in0 * f32(s0)).astype(f32)
        d = (t - in0).astype(f32)
        xh = (t - d).astype(f32)
        return (in0 - xh).astype(f32)

    _s = sq(Src0)
    _OPS["COUNT2"] = reg(
        "AWT_COUNT2",
        Spec(body=(_s <= C0) + (_s <= C1) * C2, accum=_add, accum_init=Zero,
             reference=_count2_ref))
    _OPS["CONDMIN"] = reg(
        "AWT_CONDMIN",
        Spec(body=select(sq(Src0) > C0, sq(Src0), C2), accum=minn,
             accum_init=C1, reference=_condmin_ref))
    _xh = Src0 - Src1
    _p = C0 * Src0
    _t2 = C1 * _xh - _p
    _t4 = _t2 + C1 * Src1
    _e1 = _t4 + C2 * _xh
    _OPS["E1"] = reg("AWT_E1", Spec(body=_e1, reference=_e1_ref))
    _OPS["AXPY"] = reg("AWT_AXPY", Spec(body=Src0 + C0 * Src1,
                                        reference=_axpy_ref))
    _ss = Src0 + Src1
    _bb = _ss - Src0
    _tt = (Src0 - (_ss - _bb)) + (Src1 - _bb)
    _OPS["TWOSUMT"] = reg("AWT_TWOSUMT", Spec(body=_tt, reference=_twosumt_ref))
    _t_ = Src0 * C0
    _OPS["SPLITLO"] = reg(
        "AWT_SPLITLO",
        Spec(body=Src0 - (_t_ - (_t_ - Src0)), reference=_splitlo_ref))

    def _probl_ref(in0, in1, s0, s1, imm2):
        a = (in0 + f32(s0)).astype(f32)
        c = (a >= f32(s1)).astype(f32)
        return (a - (c * f32(s1)).astype(f32)).astype(f32)

    def _probh_ref(in0, in1, s0, s1, imm2):
        a = (in0 + f32(s0)).astype(f32)
        c = (a >= f32(s1)).astype(f32)
        b = (in1 + f32(imm2)).astype(f32)
        return (b + c).astype(f32)

    _a = Src0 + C0
    _OPS["PROBL"] = reg("AWT_PROBL",
                        Spec(body=_a - (_a >= C1) * C1, reference=_probl_ref))
    _a2 = Src0 + C0
    _OPS["PROBH"] = reg("AWT_PROBH",
                        Spec(body=(Src1 + C2) + (_a2 >= C1),
                             reference=_probh_ref))

    def _paymk_ref(in0, in1, s0, s1, imm2):
        x = in0.astype(np.float32) ** 2
        idx = np.arange(in0.shape[-1], dtype=np.float32)
        body = np.where((x > s0) & (x <= s1), idx + np.float32(imm2),
                        np.float32(0.0))
        return body.astype(f32)

    _sqv = sq(Src0)
    _inpool = (_sqv > C0) & (_sqv <= C1)
    _OPS["PAYMK"] = reg("AWT_PAYMK",
                        Spec(body=select(_inpool, Idx + C2, Zero),
                             reference=_paymk_ref))

    def _selpos_ref(in0, in1, s0, s1, imm2):
        idx = np.arange(in0.shape[-1], dtype=np.float32)
        body = np.where(idx == s0, in0.astype(np.float32),
                        np.float32(0.0)).astype(f32)
        return body, body.reshape(body.shape[0], -1).sum(-1, keepdims=True)

    _OPS["SELPOS"] = reg("AWT_SELPOS",
                         Spec(body=select(eq(Idx, C0), Src0, Zero),
                              accum=_add, accum_init=Zero,
                              reference=_selpos_ref))

    def _maskfix_ref(in0, in1, s0, s1, imm2):
        fl = in1.astype(np.float32)
        x = in0.astype(np.float32)
        sv = (x * x).astype(f32)
        out = np.where(fl == np.float32(1.0), x,
                       np.where(fl == np.float32(s1), np.float32(0.0),
                                np.where(sv > s0, x, np.float32(0.0))))
        return out.astype(f32)

    _blk = select(sq(Src0) > C0, Src0, Zero)
    _OPS["MASKFIX"] = reg(
        "AWT_MASKFIX",
        Spec(body=select(eq(Src1, One), Src0, select(eq(Src1, C1), Zero, _blk)),
             reference=_maskfix_ref))
    return _OPS


# ----------------------------- device kernel --------------------------------

def build_nc(k, g, lw, debug=False):
    import concourse.bass as bass
    import concourse.tile as tile
    from concourse import bacc, mybir
    from contextlib import ExitStack

    ops = _register_ops()
    COUNT2, CONDMIN = ops["COUNT2"], ops["CONDMIN"]
    E1, AXPY, TWOSUMT = ops["E1"], ops["AXPY"], ops["TWOSUMT"]
    SPLITLO = ops["SPLITLO"]
    PROBL, PROBH = ops["PROBL"], ops["PROBH"]
    PAYMK, SELPOS, MASKFIX = ops["PAYMK"], ops["SELPOS"], ops["MASKFIX"]
    hc = host_consts()
    TS = bisect_schedule()
    k0 = int(k)
    f32 = mybir.dt.float32
    f32r = mybir.dt.float32r
    f16 = mybir.dt.float16
    i32 = mybir.dt.int32
    i16 = mybir.dt.int16
    AL = mybir.AluOpType
    bhi_, blo_ = veltkamp_split(lw)

    nc = bacc.Bacc("TRN2", debug=False, enable_asserts=False)
    x_ap = nc.dram_tensor("x", [NB, T, C], f32, kind="ExternalInput").ap()
    out_ap = nc.dram_tensor("out", [NB, T, C], f32, kind="ExternalOutput").ap()
    spill_kind = "ExternalOutput" if debug else "Internal"
    WPAD = 2080
    cad2 = [nc.dram_tensor(f"cacd_d{b}", [WPAD, 2 * C], f32,
                           kind=spill_kind).ap() for b in range(NB)]
    xed = [nc.dram_tensor(f"xed{b}", [C, TE], f32, kind=spill_kind).ap()
           for b in range(NB)]
    dbg = None
    dbg_cd = None
    dbg_win = None
    dbg_w2 = None
    dbg_of = None
    if debug:
        # rows: 0 vkf, 1 vnx, 2 t1, 3 t2, 4 ct2, 5 vke, 6 vk1e, 7 thr
        dbg = nc.dram_tensor("dbg", [8, NB, 128, NCT], f32,
                             kind="ExternalOutput").ap()
        dbg_cd = nc.dram_tensor("dbg_cd", [NB, NCT, 128, W], f32,
                                kind="ExternalOutput").ap()
        dbg_win = nc.dram_tensor("dbg_win", [NB, NCT, 128, 3 * NCAND], f32,
                                 kind="ExternalOutput").ap()
        dbg_w2 = nc.dram_tensor("dbg_w2", [NCT, 128, NCAND * FLEN], f32,
                                kind="ExternalOutput").ap()
        dbg_of = nc.dram_tensor("dbg_of", [NCT, 128, NCAND], mybir.dt.int32,
                                kind="ExternalOutput").ap()
    consts = {n: nc.dram_tensor(n.lower(), list(hc[n].shape), f32,
                                kind="ExternalInput").ap() for n in CONST_NAMES}

    with tile.TileContext(nc) as tc, ExitStack() as ctx:
        cpool = ctx.enter_context(tc.tile_pool(name="consts", bufs=1))
        xpool = ctx.enter_context(tc.tile_pool(name="x", bufs=4))
        xcpool = ctx.enter_context(tc.tile_pool(name="xc", bufs=3))
        xepool = ctx.enter_context(tc.tile_pool(name="xe", bufs=1))
        tpool = ctx.enter_context(tc.tile_pool(name="tmp", bufs=1))
        cdtp = ctx.enter_context(tc.tile_pool(name="cdt", bufs=2))
        capool = ctx.enter_context(tc.tile_pool(name="cap", bufs=2))
        cdpp = ctx.enter_context(tc.tile_pool(name="cdp", bufs=2))
        stp = ctx.enter_context(tc.tile_pool(name="state", bufs=1))
        tsp = ctx.enter_context(tc.tile_pool(name="tstage", bufs=1))
        vp = ctx.enter_context(tc.tile_pool(name="vt", bufs=2))
        wpl = ctx.enter_context(tc.tile_pool(name="win", bufs=2))
        fpl = ctx.enter_context(tc.tile_pool(name="flag", bufs=2))
        ctl = ctx.enter_context(tc.tile_pool(name="ctile", bufs=2))
        kpl = ctx.enter_context(tc.tile_pool(name="keep", bufs=1))
        dwtps = ctx.enter_context(tc.tile_pool(name="dwtps", bufs=4, space="PSUM"))
        trps = ctx.enter_context(tc.tile_pool(name="trps", bufs=2, space="PSUM"))
        idps = ctx.enter_context(tc.tile_pool(name="idps", bufs=1, space="PSUM"))

        ct = {}
        for name in CONST_NAMES:
            t_ = cpool.tile(list(hc[name].shape), f32, tag=name)
            nc.sync.dma_start(t_[:], consts[name][:])
            ct[name] = t_

        # per-channel-tile flat row base for window gathers: (cb*128+p)*TE
        fbf = []
        for cb in range(NCT):
            fbi = cpool.tile([128, 1], i32, tag=f"fbi{cb}")
            nc.gpsimd.iota(fbi[:], pattern=[[1, 1]], base=cb * 128 * TE,
                           channel_multiplier=TE)
            fbf_ = cpool.tile([128, 1], f32, tag=f"fbf{cb}")
            nc.vector.tensor_copy(fbf_[:], fbi[:])
            fbf.append(fbf_)

        zt = cpool.tile([32, C], f32, tag="ZPAD", name="ZPAD")
        nc.vector.memset(zt[:], 0.0)

        def st(tag, dtype=f32, cols=NCT):
            return stp.tile([128, cols], dtype, tag=tag, name=tag)

        def ti(tag, cols=NCAND, dtype=f32):
            return ctl.tile([128, cols], dtype, tag=tag, name=tag)

        # ------------- P1: PE ca+cd matmuls; ca spill; cd -> channel-major ---
        def emit_p1(b):
            cdts = [cdtp.tile([128, W], f32, tag=f"cd{cb}", name=f"cd{cb}")
                    for cb in range(NCT)]
            xt = {}
            capair = None
            cdpair = None
            for o in range(33):
                if o < 32:
                    xt[o] = xpool.tile([128, C], f32, tag="x", name="x")
                    nc.sync.dma_start(xt[o][:],
                                      x_ap[b, 128 * o:128 * o + 128, :])
                    ps = dwtps.tile([128, C], f32, tag="dwt", name="dwt")
                    mb = ct["MB0"] if o == 0 else ct["MBi"]
                    nc.tensor.matmul(ps[:], mb[:], xt[o][:],
                                     start=True, stop=(o == 0))
                    if o > 0:
                        nc.tensor.matmul(ps[:], ct["MAi"][64:128, :],
                                         xt[o - 1][64:128, :],
                                         start=False, stop=True)
                        xt.pop(o - 1)
                    nrow = 64
                else:
                    ps = dwtps.tile([128, C], f32, tag="dwt", name="dwt")
                    nc.tensor.matmul(ps[0:71, :], ct["MT"][:], xt[31][:],
                                     start=True, stop=True)
                    nrow = 7
                if o % 2 == 0:
                    capair = capool.tile([128, C], f32, tag="cat", name="cat")
                    cdpair = cdpp.tile([128, C], f32, tag="cdq", name="cdq")
                r = 64 * (o % 2)
                nc.scalar.copy(capair[r:r + nrow, :], ps[0:nrow, :])
                nc.scalar.copy(cdpair[r:r + nrow, :], ps[64:64 + nrow, :])
                if o % 2 == 1 or o == 32:
                    w_ = o // 2
                    nrows = 128 if w_ < 16 else W - 2048
                    nc.scalar.dma_start(
                        cad2[b][128 * w_:128 * w_ + nrows, 0:C],
                        capair[0:nrows, :])
                    for cb in range(NCT):
                        tp = trps.tile([128, 128], f32, tag="tr", name="tr")
                        nc.tensor.transpose(
                            tp[:, 0:nrows],
                            cdpair[0:nrows, 128 * cb:128 * cb + 128],
                            ct["IDENT"][0:nrows, 0:nrows])
                        nc.scalar.copy(
                            cdts[cb][:, 128 * w_:128 * w_ + nrows],
                            tp[:, 0:nrows])
            # zero the pad rows (W..WPAD) of both halves once per batch
            nc.sync.dma_start(cad2[b][W:WPAD, 0:C], zt[0:WPAD - W, :])
            nc.scalar.dma_start(cad2[b][W:WPAD, C:2 * C], zt[0:WPAD - W, :])
            return cdts

        # ------------- P1b: build xT_ext for a (b, cb-pair); spill to DRAM ---
        def emit_build_xe_pair(b, cbs):
            xes = [xepool.tile([128, TE], f32, tag=f"xe{cb % 2}",
                               name=f"xe{cb % 2}") for cb in cbs]
            for o in range(32):
                xc = xcpool.tile([128, 256], f32, tag="xc", name="xc")
                nc.sync.dma_start(
                    xc[:], x_ap[b, 128 * o:128 * o + 128,
                                128 * cbs[0]:128 * cbs[0] + 256])
                for i in range(2):
                    tp = trps.tile([128, 128], f32, tag="tr", name="tr")
                    nc.tensor.transpose(tp[:], xc[:, 128 * i:128 * i + 128],
                                        ct["IDENT"][:])
                    nc.scalar.copy(xes[i][:, 15 + 128 * o:15 + 128 * o + 128],
                                   tp[:])
            # mirror columns: head m=0..14 <- col 29-m ; tail 4111+i <- 4110-i
            for xe in xes:
                for m in range(15):
                    nc.scalar.copy(xe[:, m:m + 1], xe[:, 29 - m:30 - m])
                    nc.scalar.copy(xe[:, 4111 + m:4112 + m],
                                   xe[:, 4110 - m:4111 - m])
            for i, cb in enumerate(cbs):
                nc.sync.dma_start(xed[b][128 * cb:128 * cb + 128, :],
                                  xes[i][:])

        # ------------- P2a: per-batch bisection on approx cd^2 ---------------
        def emit_bisect(b, cdts):
            lol, loh = st(f"lol{b}"), st(f"loh{b}")
            nc.vector.memset(lol[:], 0.0)
            nc.vector.memset(loh[:], 0.0)
            comb = st("comb")
            c1t, e1c, e2c, sf = st("c1t"), st("e1c"), st("e2c"), st("sf")
            tmp, tmp2, carry = st("tmp"), st("tmp2"), st("carry")
            ci = st("ci", i32)
            k4095 = st("k4095", i32)
            nc.vector.memset(k4095[:], 4095)
            k65535 = st("k65535", i32)
            nc.vector.memset(k65535[:], 65535)
            m1l, m1h, m2l, m2h = st("m1l"), st("m1h"), st("m2l"), st("m2h")
            m1i, m2i = st("m1i", i32), st("m2i", i32)
            ih = st("ih", i32)
            kp1 = float(k0 + 1)

            def mk_probe(ml, mh, mi, off):
                offl, offh = float(off & 0xFFFF), float(off >> 16)
                nc.vector._custom_dve(PROBL, out=ml[:], in0=lol[:],
                                      s0=offl, s1=65536.0)
                nc.vector._custom_dve(PROBH, out=mh[:], in0=lol[:],
                                      in1=loh[:], s0=offl, s1=65536.0,
                                      imm2=offh)
                nc.vector.tensor_copy(ih[:], mh[:])
                nc.vector.tensor_scalar(out=ih[:], in0=ih[:], scalar1=16,
                                        scalar2=None,
                                        op0=AL.logical_shift_left)
                nc.vector.tensor_copy(mi[:], ml[:])
                nc.vector.tensor_tensor(out=mi[:], in0=mi[:], in1=ih[:],
                                        op=AL.bitwise_or)

            for Tstep in TS[:LVL]:
                mk_probe(m1l, m1h, m1i, Tstep - 1)
                mk_probe(m2l, m2h, m2i, 2 * Tstep - 1)
                m1f = m1i[:].bitcast(f32)
                m2f = m2i[:].bitcast(f32)
                for cb in range(NCT):
                    sc = tpool.tile([128, W], f32, tag="p", name="cscr")
                    nc.vector._custom_dve(
                        COUNT2, out=sc[:], accum_out=comb[:, cb:cb + 1],
                        in0=cdts[cb][:], s0=m1f[:, cb:cb + 1],
                        s1=m2f[:, cb:cb + 1], imm2=4096.0)
                nc.vector.tensor_copy(ci[:], comb[:])
                nc.vector.tensor_tensor(out=ci[:], in0=ci[:], in1=k4095[:],
                                        op=AL.bitwise_and)
                nc.vector.tensor_copy(c1t[:], ci[:])
                nc.vector.tensor_scalar(out=e1c[:], in0=c1t[:], scalar1=kp1,
                                        scalar2=None, op0=AL.is_lt)
                nc.vector.tensor_scalar(out=e2c[:], in0=comb[:],
                                        scalar1=4096.0 * kp1, scalar2=None,
                                        op0=AL.is_lt)
                nc.vector.tensor_tensor(out=sf[:], in0=e1c[:], in1=e2c[:],
                                        op=AL.add)
                tl_, th_ = float(Tstep & 0xFFFF), float(Tstep >> 16)
                nc.vector.tensor_scalar(out=tmp[:], in0=sf[:], scalar1=tl_,
                                        scalar2=None, op0=AL.mult)
                nc.vector.tensor_tensor(out=lol[:], in0=lol[:], in1=tmp[:],
                                        op=AL.add)
                nc.vector.tensor_copy(ci[:], lol[:])
                nc.vector.tensor_tensor(out=ci[:], in0=ci[:], in1=k65535[:],
                                        op=AL.bitwise_and)
                nc.vector.tensor_copy(tmp2[:], ci[:])
                nc.vector.tensor_tensor(out=carry[:], in0=lol[:], in1=tmp2[:],
                                        op=AL.subtract)
                nc.vector.tensor_scalar(out=carry[:], in0=carry[:],
                                        scalar1=1.0 / 65536.0, scalar2=None,
                                        op0=AL.mult)
                nc.vector.tensor_copy(lol[:], tmp2[:])
                if th_ != 0.0:
                    nc.vector.tensor_scalar(out=tmp[:], in0=sf[:], scalar1=th_,
                                            scalar2=None, op0=AL.mult)
                    nc.vector.tensor_tensor(out=loh[:], in0=loh[:],
                                            in1=tmp[:], op=AL.add)
                nc.vector.tensor_tensor(out=loh[:], in0=loh[:], in1=carry[:],
                                        op=AL.add)

            vk = st(f"vk{b}", i32)
            nc.vector.tensor_copy(ih[:], loh[:])
            nc.vector.tensor_scalar(out=ih[:], in0=ih[:], scalar1=16,
                                    scalar2=None, op0=AL.logical_shift_left)
            nc.vector.tensor_copy(vk[:], lol[:])
            nc.vector.tensor_tensor(out=vk[:], in0=vk[:], in1=ih[:],
                                    op=AL.bitwise_or)
            vkf = vk[:].bitcast(f32)
            # hi = lo + d_rem - 1 in bit space (count(<=hi) >= k0+1 invariant)
            d_rem = TS[LVL - 1]
            mk_probe(m1l, m1h, m1i, d_rem - 1)
            hif = m1i[:].bitcast(f32)

            # vnext = min approx^2 strictly above hi ; band (T1, T2]
            vnx = st(f"vnx{b}")
            for cb in range(NCT):
                sc2 = tpool.tile([128, W], f32, tag="e", name="cscr2")
                nc.vector._custom_dve(
                    CONDMIN, out=sc2[:], accum_out=vnx[:, cb:cb + 1],
                    in0=cdts[cb][:], s0=hif[:, cb:cb + 1], s1=3.0e38,
                    imm2=3.0e38)
            t1t, t2t = st(f"t1_{b}"), st(f"t2_{b}")
            nc.vector.tensor_scalar(out=t1t[:], in0=vkf, scalar1=1.0 - DELTA,
                                    scalar2=None, op0=AL.mult)
            nc.vector.tensor_scalar(out=t2t[:], in0=vnx[:], scalar1=1.0 + DELTA,
                                    scalar2=None, op0=AL.mult)
            # cT2 = count(approx^2 <= T2); descending select positions
            comb2 = st("comb2")
            for cb in range(NCT):
                sc = tpool.tile([128, W], f32, tag="p", name="cscr")
                nc.vector._custom_dve(
                    COUNT2, out=sc[:], accum_out=comb2[:, cb:cb + 1],
                    in0=cdts[cb][:], s0=t1t[:, cb:cb + 1],
                    s1=t2t[:, cb:cb + 1], imm2=4096.0)
            ci2 = st("ci2", i32)
            ct1f = st("ct1f")
            nc.vector.tensor_copy(ci2[:], comb2[:])
            nc.vector.tensor_tensor(out=ci2[:], in0=ci2[:], in1=k4095[:],
                                    op=AL.bitwise_and)
            nc.vector.tensor_copy(ct1f[:], ci2[:])
            ct2f = st(f"ct2_{b}")
            nc.vector.tensor_tensor(out=ct2f[:], in0=comb2[:], in1=ct1f[:],
                                    op=AL.subtract)
            nc.vector.tensor_scalar(out=ct2f[:], in0=ct2f[:],
                                    scalar1=1.0 / 4096.0, scalar2=None,
                                    op0=AL.mult)
            dp1, dp2 = st(f"dp1_{b}"), st(f"dp2_{b}")
            nc.vector.tensor_scalar(out=dp1[:], in0=ct2f[:],
                                    scalar1=-float(k0 + 1), scalar2=None,
                                    op0=AL.add)
            nc.vector.tensor_scalar(out=dp2[:], in0=ct2f[:],
                                    scalar1=-float(k0 + 2), scalar2=None,
                                    op0=AL.add)
            return vkf, t1t, t2t, dp1, dp2, ct2f, ct1f

        # ------------- P2b: candidate extraction + exact chains --------------
        def emit_cand_extract(b, cb, cdts, t1t, t2t, keep):
            scr = tpool.tile([128, W], f32, tag="p", name="cscr")
            scr2 = tpool.tile([128, W], f32, tag="e", name="cscr2")
            nc.vector._custom_dve(PAYMK, out=scr[:], in0=cdts[cb][:],
                                  s0=t1t[:, cb:cb + 1], s1=t2t[:, cb:cb + 1],
                                  imm2=4096.0)
            pay = keep["pay"]
            nc.vector.max(pay[:, 0:8], scr[:])
            nc.vector.match_replace(scr2[:], pay[:, 0:8], scr[:], 0.0)
            nc.vector.max(pay[:, 8:16], scr2[:])
            validf, idxf = keep["val"], keep["idx"]
            nc.vector.tensor_scalar(out=validf[:], in0=pay[:], scalar1=4096.0,
                                    scalar2=None, op0=AL.is_ge)
            nc.vector.tensor_scalar(out=idxf[:], in0=pay[:], scalar1=-4096.0,
                                    scalar2=None, op0=AL.add)
            offf = ti("offf")
            nc.vector.tensor_scalar(out=offf[:], in0=idxf[:], scalar1=2.0,
                                    scalar2=None, op0=AL.mult)
            nc.vector.tensor_scalar(out=offf[:], in0=offf[:], scalar1=1.0,
                                    scalar2=None, op0=AL.add)
            nc.vector.tensor_tensor(out=offf[:], in0=offf[:],
                                    in1=fbf[cb][:].to_broadcast([128, NCAND]),
                                    op=AL.add)
            nc.vector.tensor_tensor(out=offf[:], in0=offf[:], in1=validf[:],
                                    op=AL.mult)
            offi = ti("offi", NCAND, i32)
            nc.vector.tensor_copy(offi[:], offf[:])
            win = wpl.tile([128, NCAND * FLEN], f32, tag=f"win{cb}",
                           name=f"win{cb}")
            for c_ in range(NCAND):
                nc.gpsimd.indirect_dma_start(
                    out=win[:, 16 * c_:16 * c_ + 16], out_offset=None,
                    in_=xed[b][:, :],
                    in_offset=bass.IndirectOffsetOnAxis(
                        ap=offi[:, c_:c_ + 1], axis=1))
            if debug and b == 0:
                nc.sync.dma_start(dbg_w2[cb], win[:])
                nc.sync.dma_start(dbg_of[cb], offi[:])
            winl = wpl.tile([128, NCAND * FLEN], f32, tag=f"winl{cb}",
                            name=f"winl{cb}")
            nc.vector._custom_dve(SPLITLO, out=winl[:], in0=win[:], s0=4097.0)
            keep["win"], keep["winl"] = win, winl

        def wsl(t_, j):
            return t_[:, j:j + 16 * (NCAND - 1) + 1:16]

        def emit_chains(b, keeps):
            # 4 channel tiles interleaved per tap: Pool does p/s/r, DVE e/t/y
            cur = {}
            nxt = {}
            pt, et, st_, tt = {}, {}, {}, {}
            for cb in range(NCT):
                cur[cb] = ti(f"ya{cb}")
                nxt[cb] = ti(f"yb{cb}")
                pt[cb] = ti(f"pc{cb}")
                et[cb] = ti(f"ec{cb}")
                st_[cb] = ti(f"sc_{cb}")
                tt[cb] = ti(f"tc_{cb}")
                nc.gpsimd.tensor_scalar(out=cur[cb][:],
                                        in0=wsl(keeps[cb]["win"], 0),
                                        scalar1=float(KHI[0]), scalar2=None,
                                        op0=AL.mult)
            for j in range(1, FLEN):
                h = float(KHI[j])
                hh, hl = veltkamp_split(KHI[j])
                for cb in range(NCT):
                    nc.gpsimd.tensor_scalar(out=pt[cb][:],
                                            in0=wsl(keeps[cb]["win"], j),
                                            scalar1=h, scalar2=None,
                                            op0=AL.mult)
                for cb in range(NCT):
                    nc.vector._custom_dve(E1, out=et[cb][:],
                                          in0=wsl(keeps[cb]["win"], j),
                                          in1=wsl(keeps[cb]["winl"], j),
                                          s0=h, s1=hh, imm2=hl)
                for cb in range(NCT):
                    nc.vector._custom_dve(AXPY, out=et[cb][:], in0=et[cb][:],
                                          in1=wsl(keeps[cb]["winl"], j), s0=hl)
                for cb in range(NCT):
                    nc.gpsimd.tensor_tensor(out=st_[cb][:], in0=pt[cb][:],
                                            in1=cur[cb][:], op=AL.add)
                for cb in range(NCT):
                    nc.vector._custom_dve(TWOSUMT, out=tt[cb][:],
                                          in0=pt[cb][:], in1=cur[cb][:])
                for cb in range(NCT):
                    nc.gpsimd.tensor_tensor(out=tt[cb][:], in0=tt[cb][:],
                                            in1=et[cb][:], op=AL.add)
                for cb in range(NCT):
                    nc.vector.tensor_tensor(out=nxt[cb][:], in0=st_[cb][:],
                                            in1=tt[cb][:], op=AL.add)
                    cur[cb], nxt[cb] = nxt[cb], cur[cb]
            return cur

        def emit_cand_sel(b, cb, ycur, dp1, dp2, vke, vk1e, keep):
            validf = keep["val"]
            xsq = keep["xsq"]
            nc.vector.tensor_tensor(out=xsq[:], in0=ycur[:], in1=ycur[:],
                                    op=AL.mult)
            a_ = ti("av")
            nc.vector.tensor_scalar(out=a_[:], in0=xsq[:], scalar1=1.0,
                                    scalar2=None, op0=AL.add)
            nc.vector.tensor_tensor(out=a_[:], in0=a_[:], in1=validf[:],
                                    op=AL.mult)
            nc.vector.tensor_scalar(out=a_[:], in0=a_[:], scalar1=-1.0,
                                    scalar2=None, op0=AL.add)
            srt, scr3 = ti("srt"), ti("scr3")
            nc.vector.max(srt[:, 0:8], a_[:])
            nc.vector.match_replace(scr3[:], srt[:, 0:8], a_[:], -2.0)
            nc.vector.max(srt[:, 8:16], scr3[:])
            dump = ti("dump")
            nc.vector._custom_dve(SELPOS, out=dump[:],
                                  accum_out=vke[:, cb:cb + 1], in0=srt[:],
                                  s0=dp1[:, cb:cb + 1])
            nc.vector._custom_dve(SELPOS, out=dump[:],
                                  accum_out=vk1e[:, cb:cb + 1], in0=srt[:],
                                  s0=dp2[:, cb:cb + 1])

        # ------------- P2c: thr = fma(vk, lw, RN(vh*g)) via Dekker -----------
        def emit_thr(b, vke, vk1e):
            thr = st(f"thr{b}")
            if g == 0.0:
                nc.vector.tensor_copy(thr[:], vke[:])
                return thr
            tmp = st("tmp")
            cc, tt2, ah, al2 = st("cc"), st("tt2"), st("ah"), st("al2")
            ph, er, ss_, bb, t1 = (st("ph"), st("er"), st("ss_"), st("bb"),
                                   st("t1"))
            vkap = vke[:]
            nc.vector.tensor_scalar(out=cc[:], in0=vk1e[:], scalar1=float(g),
                                    scalar2=None, op0=AL.mult)
            nc.vector.tensor_scalar(out=tt2[:], in0=vkap, scalar1=4097.0,
                                    scalar2=None, op0=AL.mult)
            nc.vector.tensor_tensor(out=ah[:], in0=tt2[:], in1=vkap,
                                    op=AL.subtract)
            nc.vector.tensor_tensor(out=ah[:], in0=tt2[:], in1=ah[:],
                                    op=AL.subtract)
            nc.vector.tensor_tensor(out=al2[:], in0=vkap, in1=ah[:],
                                    op=AL.subtract)
            nc.vector.tensor_scalar(out=ph[:], in0=vkap, scalar1=float(lw),
                                    scalar2=None, op0=AL.mult)
            nc.vector.tensor_scalar(out=er[:], in0=ah[:], scalar1=bhi_,
                                    scalar2=None, op0=AL.mult)
            nc.vector.tensor_tensor(out=er[:], in0=er[:], in1=ph[:],
                                    op=AL.subtract)
            for a_, b_ in ((ah, blo_), (al2, bhi_), (al2, blo_)):
                nc.vector.tensor_scalar(out=tmp[:], in0=a_[:], scalar1=b_,
                                        scalar2=None, op0=AL.mult)
                nc.vector.tensor_tensor(out=er[:], in0=er[:], in1=tmp[:],
                                        op=AL.add)
            nc.vector.tensor_tensor(out=ss_[:], in0=ph[:], in1=cc[:], op=AL.add)
            nc.vector.tensor_tensor(out=bb[:], in0=ss_[:], in1=ph[:],
                                    op=AL.subtract)
            nc.vector.tensor_tensor(out=t1[:], in0=cc[:], in1=bb[:],
                                    op=AL.subtract)
            nc.vector.tensor_tensor(out=bb[:], in0=ss_[:], in1=bb[:],
                                    op=AL.subtract)
            nc.vector.tensor_tensor(out=bb[:], in0=ph[:], in1=bb[:],
                                    op=AL.subtract)
            nc.vector.tensor_tensor(out=t1[:], in0=t1[:], in1=bb[:], op=AL.add)
            nc.vector.tensor_tensor(out=t1[:], in0=t1[:], in1=er[:], op=AL.add)
            nc.vector.tensor_tensor(out=thr[:], in0=ss_[:], in1=t1[:],
                                    op=AL.add)
            return thr

        # ------------- P2d: flags scatter + fused mask pass ------------------
        def emit_cand_c(b, cb, cdts, thr, vkf, keep):
            validf, idxf, xsq = keep["val"], keep["idx"], keep["xsq"]
            bit, flagf = ti("bit"), ti("flagf")
            nc.vector.tensor_tensor(
                out=bit[:], in0=thr[:, cb:cb + 1].to_broadcast([128, NCAND]),
                in1=xsq[:], op=AL.is_lt)
            nc.vector.tensor_scalar(out=flagf[:], in0=bit[:], scalar1=-1.0,
                                    scalar2=None, op0=AL.mult)
            nc.vector.tensor_scalar(out=flagf[:], in0=flagf[:], scalar1=2.0,
                                    scalar2=None, op0=AL.add)
            flv = ti("flv", NCAND, f16)
            nc.vector.tensor_copy(flv[:], flagf[:])
            c1_, m1_, ls_ = ti("c1_"), ti("m1_"), ti("ls_")
            ls1 = ti("ls1", NCAND, i16)
            ls2 = ti("ls2", NCAND, i16)
            # half 1: idx in [0,1024) -> idx else -1
            nc.vector.tensor_scalar(out=c1_[:], in0=idxf[:], scalar1=1024.0,
                                    scalar2=None, op0=AL.is_lt)
            nc.vector.tensor_tensor(out=m1_[:], in0=c1_[:], in1=validf[:],
                                    op=AL.mult)
            nc.vector.tensor_scalar(out=ls_[:], in0=idxf[:], scalar1=1.0,
                                    scalar2=None, op0=AL.add)
            nc.vector.tensor_tensor(out=ls_[:], in0=ls_[:], in1=m1_[:],
                                    op=AL.mult)
            nc.vector.tensor_scalar(out=ls_[:], in0=ls_[:], scalar1=-1.0,
                                    scalar2=None, op0=AL.add)
            nc.vector.tensor_copy(ls1[:], ls_[:])
            # half 2: idx in [1024, 2055) -> idx-1024 else -1
            nc.vector.tensor_scalar(out=c1_[:], in0=idxf[:], scalar1=1024.0,
                                    scalar2=None, op0=AL.is_ge)
            nc.vector.tensor_tensor(out=m1_[:], in0=c1_[:], in1=validf[:],
                                    op=AL.mult)
            nc.vector.tensor_scalar(out=ls_[:], in0=idxf[:], scalar1=-1023.0,
                                    scalar2=None, op0=AL.add)
            nc.vector.tensor_tensor(out=ls_[:], in0=ls_[:], in1=m1_[:],
                                    op=AL.mult)
            nc.vector.tensor_scalar(out=ls_[:], in0=ls_[:], scalar1=-1.0,
                                    scalar2=None, op0=AL.add)
            nc.vector.tensor_copy(ls2[:], ls_[:])
            flag = fpl.tile([128, 2056], f16, tag="fl", name="fl")
            nc.gpsimd.local_scatter(flag[:, 0:1024], flv[:], ls1[:],
                                    128, 1024, NCAND)
            nc.gpsimd.local_scatter(flag[:, 1024:2056], flv[:], ls2[:],
                                    128, 1032, NCAND)
            nc.vector._custom_dve(MASKFIX, out=cdts[cb][:], in0=cdts[cb][:],
                                  in1=flag[:, 0:W], s0=vkf[:, cb:cb + 1],
                                  s1=2.0)

        # ------------- P2e: masked cd transpose + spill ----------------------
        def emit_mask_spill(b, cdts):
            for wb in range(17):
                cc = min(128, W - 128 * wb)
                stg = tsp.tile([128, C], f32, tag="ts", name="ts")
                for cb in range(NCT):
                    tp = trps.tile([128, 128], f32, tag="tr", name="tr")
                    nc.tensor.transpose(tp[0:cc, :],
                                        cdts[cb][:, 128 * wb:128 * wb + cc],
                                        ct["IDENT"][:])
                    nc.scalar.copy(stg[0:cc, 128 * cb:128 * cb + 128],
                                   tp[0:cc, :])
                nc.scalar.dma_start(
                    cad2[b][128 * wb:128 * wb + cc, C:2 * C], stg[0:cc, :])

        # ------------- P3: IDWT, per batch (2 output chunks per iter) --------
        def emit_p3(b):
            rl = ct["RL_A"][:]
            rh = ct["RH_A"][:]
            for vp_ in range(32):
                r0 = 64 * vp_
                vta = vp.tile([64, 2 * C], f32, tag="vt", name="vt")
                nc.sync.dma_start(vta[:], cad2[b][r0:r0 + 64, :])
                vtb = vp.tile([64, 2 * C], f32, tag="vt2", name="vt2")
                nc.sync.dma_start(vtb[:], cad2[b][r0 + 32:r0 + 96, :])
                psa = idps.tile([64, C], f32, tag="idwta", name="idwta")
                nc.tensor.matmul(psa[:], rl, vta[:, 0:C],
                                 start=True, stop=False)
                nc.tensor.matmul(psa[:], rh, vta[:, C:2 * C],
                                 start=False, stop=True)
                psb = idps.tile([64, C], f32, tag="idwtb", name="idwtb")
                nc.tensor.matmul(psb[:], rl, vtb[:, 0:C],
                                 start=True, stop=False)
                nc.tensor.matmul(psb[:], rh, vtb[:, C:2 * C],
                                 start=False, stop=True)
                ot = vp.tile([128, C], f32, tag="ot", name="ot")
                nc.scalar.copy(ot[0:64, :], psa[:])
                nc.scalar.copy(ot[64:128, :], psb[:])
                nc.scalar.dma_start(out_ap[b, 128 * vp_:128 * vp_ + 128, :],
                                    ot[:])

        # ----------------------------- schedule ------------------------------
        keeps = [{nm: kpl.tile([128, NCAND], f32, tag=f"{nm}{cb}",
                               name=f"{nm}{cb}")
                  for nm in ("pay", "val", "idx", "xsq")}
                 for cb in range(NCT)]
        cdts_cur = emit_p1(0)
        emit_build_xe_pair(0, (0, 1))
        emit_build_xe_pair(0, (2, 3))
        for b in range(NB):
            vkf, t1t, t2t, dp1, dp2, ct2f, ct1f = emit_bisect(b, cdts_cur)
            vke, vk1e = st(f"vke{b}"), st(f"vk1e{b}")
            if debug:
                for cb in range(NCT):
                    nc.sync.dma_start(dbg_cd[b, cb], cdts_cur[cb][:])
            for cb in range(NCT):
                emit_cand_extract(b, cb, cdts_cur, t1t, t2t, keeps[cb])
            ycur = emit_chains(b, keeps)
            for cb in range(NCT):
                emit_cand_sel(b, cb, ycur[cb], dp1, dp2, vke, vk1e, keeps[cb])
            thr = emit_thr(b, vke, vk1e)
            if debug:
                nc.sync.dma_start(dbg[0, b], vkf)
                nc.sync.dma_start(dbg[4, b], ct2f[:])
                nc.sync.dma_start(dbg[1, b], ct1f[:])
                nc.sync.dma_start(dbg[2, b], t1t[:])
                nc.sync.dma_start(dbg[3, b], t2t[:])
                nc.sync.dma_start(dbg[5, b], vke[:])
                nc.sync.dma_start(dbg[6, b], vk1e[:])
                nc.sync.dma_start(dbg[7, b], thr[:])
                for cb in range(NCT):
                    kp = keeps[cb]
                    nc.sync.dma_start(dbg_win[b, cb, :, 0:NCAND], kp["pay"][:])
                    nc.sync.dma_start(dbg_win[b, cb, :, NCAND:2 * NCAND],
                                      kp["idx"][:])
                    nc.sync.dma_start(dbg_win[b, cb, :, 2 * NCAND:3 * NCAND],
                                      kp["xsq"][:])
            for cb in range(NCT):
                emit_cand_c(b, cb, cdts_cur, thr, vkf, keeps[cb])
            cdts_next = None
            if b + 1 < NB:
                cdts_next = emit_p1(b + 1)
                emit_build_xe_pair(b + 1, (0, 1))
                emit_build_xe_pair(b + 1, (2, 3))
            emit_mask_spill(b, cdts_cur)
            emit_p3(b)
            cdts_cur = cdts_next

    nc.compile()
    return nc


_NC_CACHE = {}


def kernel(x_in: np.ndarray, threshold_param: np.ndarray) -> np.ndarray:
    from concourse import bass_utils
    q = np.float32(np.asarray(threshold_param).reshape(-1)[0])
    k, g, lw = quantile_host_params(q)
    key = (k, g, lw)
    if key not in _NC_CACHE:
        _NC_CACHE[key] = build_nc(k, g, lw)
    nc = _NC_CACHE[key]
    hc = host_consts()
    cmaps = {n.lower(): hc[n] for n in CONST_NAMES}
    x_in = np.ascontiguousarray(x_in, dtype=np.float32)
    in_maps = [{"x": x_in[NB * c:NB * (c + 1)], **cmaps} for c in range(NCORES)]
    res = bass_utils.run_bass_kernel_spmd(nc, in_maps,
                                          core_ids=list(range(NCORES)))
    return np.concatenate([res.results[c]["out"] for c in range(NCORES)],
                          axis=0)
